# revision 29
# baseline (speedup 1.0000x reference)
"""Trainium2 Bass kernel for nn_AttnFreeLayer (linear-attention-style layer).

Computes, for inputs q,k,v [B,S,D] and weights Wq,Wk,Wv [E,D] (E=D):
    q_in = elu(q @ Wq^T) + 1
    k_in = elu(k @ Wk^T) + 1
    v_in = v @ Wv^T
    kv_in = k_in * v_in
    out = q_in * (kv_in + cumsum_s(kv_in)) / cumsum_s(k_in)

Sharding: 8 cores = 4 batches x 2 halves of the output dim E. Each core
computes out[b, :, e0:e0+512] from the full q/k/v of its batch and the
matching W^T slices. The seq-cumsum is core-local (it runs along seq
independently per output dim), so no collectives are needed.

Host prep: q/k/v are transposed to [D, S] and cast to fp16 on the host
(once per batch, shared by the core pair). This makes the device-side
loads plain contiguous 2 MB DMAs with the contraction dim on partitions
(measured ~330 GB/s vs ~80 GB/s for on-device DMA-transpose of the
natural layout) and halves HBM traffic vs fp32.

Per-core kernel (natural [s, e] layout, 64 blocks of 128 seq rows):
  - projections: fp16 matmuls, x^T 128x128 chunk stationary, W^T moving,
    fp32 PSUM accumulate over 8 contraction chunks
  - elu(x)+1 = exp(-relu(-x)) + relu(x)  (ACT Relu/Exp + DVE add)
  - cumsum per block: upper-triangular-ones matmul (block-local inclusive
    cumsum) + a carry-broadcast matmul into the same PSUM. The carry is
    row 127 of the previous block's inclusive cumsum: its PSUM tail rows
    [96:128] are copied to SBUF (partition base 96 is access-legal where
    127 alone is not) and a constant selector matrix (row 127 all-ones)
    extracts/broadcasts it as a K=32 matmul at tile_position (96, 0)
  - combine: reciprocal + adds/muls on DVE, store fp32
  - 1-block software-pipeline skew keeps the PE dense

Measured (R=129 repeat-loop delta): ~520-550 us/pass per core; modeled
PE busy ~381 us (24 projection matmuls + 4 cumsum matmuls per block at
213 ns each). Loads (~154 us), ACT (~313 us), DVE (~278 us) all hide
under the PE stream.
"""

import sys

for _p in ("/opt/trn_rl_repo",):
    if _p not in sys.path:
        sys.path.insert(0, _p)

from contextlib import ExitStack

import numpy as np

import concourse.bass as bass
import concourse.tile as tile
from concourse import bacc
from concourse import mybir
from concourse.bass_utils import run_bass_kernel_spmd

FP16 = mybir.dt.float16
FP32 = mybir.dt.float32
AF = mybir.ActivationFunctionType

B, S, D, E = 4, 8192, 1024, 1024
NCORES = 8
EH = E // 2  # e-half per core
P = 128  # block size (partitions)
CHUNK = 1024  # seq rows per transposed-load chunk


def build_nc(S=S, D=D, EH=EH, CHUNK=CHUNK, debug=False, repeat=1,
             pp_bufs=4, pc_bufs=3, a_bufs=3, x_bufs=None, skew=1):
    n_d = D // P
    n_chunks = S // CHUNK
    blocks_per_chunk = CHUNK // P

    nc = bacc.Bacc("TRN2", target_bir_lowering=False, debug=debug)

    xq = nc.declare_dram_parameter("xq", [D, S], FP16, isOutput=False)
    xk = nc.declare_dram_parameter("xk", [D, S], FP16, isOutput=False)
    xv = nc.declare_dram_parameter("xv", [D, S], FP16, isOutput=False)
    wqt = nc.declare_dram_parameter("wqt", [D, EH], FP16, isOutput=False)
    wkt = nc.declare_dram_parameter("wkt", [D, EH], FP16, isOutput=False)
    wvt = nc.declare_dram_parameter("wvt", [D, EH], FP16, isOutput=False)
    tri = nc.declare_dram_parameter("tri", [P, P], FP16, isOutput=False)
    sel = nc.declare_dram_parameter("sel", [P, P], FP16, isOutput=False)
    out = nc.declare_dram_parameter("out", [S, EH], FP32, isOutput=True)

    with tile.TileContext(nc) as tc, ExitStack() as ctx:
        wpool = ctx.enter_context(tc.tile_pool(name="w", bufs=3))
        xpool = ctx.enter_context(tc.tile_pool(name="xT", bufs=x_bufs or 2))
        apool = ctx.enter_context(tc.tile_pool(name="act", bufs=a_bufs))
        cpool = ctx.enter_context(tc.tile_pool(name="carry", bufs=4))
        opool = ctx.enter_context(tc.tile_pool(name="out", bufs=3))
        pp = ctx.enter_context(tc.tile_pool(name="pproj", bufs=pp_bufs, space="PSUM"))
        pc = ctx.enter_context(tc.tile_pool(name="pcum", bufs=pc_bufs, space="PSUM"))

        # --- weights / constants (resident) ---
        # one DMA per weight matrix: [D, EH] -> [128, n_d*EH] (d-chunk major)
        w_tiles = []  # [input][d_chunk] -> [P, EH] fp16 views
        for wd in (wqt, wkt, wvt):
            wt = wpool.tile([P, n_d * EH], FP16, tag="wbig")
            nc.sync.dma_start(
                out=wt[:].rearrange("p (j e) -> p j e", j=n_d),
                in_=wd[:].rearrange("(j p) e -> p j e", p=P),
            )
            w_tiles.append([wt[:, j * EH : (j + 1) * EH] for j in range(n_d)])
        tri_t = wpool.tile([P, P], FP16, tag="w")
        nc.sync.dma_start(out=tri_t[:], in_=tri[:])
        # selector: row 127 all-ones, rest zero; used as lhsT[96:128, :] to
        # extract PSUM row 127 (the inclusive block total) via a K=32 matmul
        # at tile_position (96, 0) - partition base 96 is access-legal,
        # 127 alone is not.
        sel_t = wpool.tile([P, P], FP16, tag="w")
        nc.sync.dma_start(out=sel_t[:], in_=sel[:])
        n_blocks_total = n_chunks * blocks_per_chunk

        def emit_proj(xT, s0):
            """Projection matmuls for one 128-row block at col offset s0
            within the current chunk's x^T tiles. Returns (psq, psk, psv)."""
            ps = []
            for i in range(3):
                p = pp.tile([P, EH], FP32, tag="proj")
                for j in range(n_d):
                    nc.tensor.matmul(
                        p[:],
                        lhsT=xT[i][:, j, s0 : s0 + P],
                        rhs=w_tiles[i][j],
                        start=(j == 0),
                        stop=(j == n_d - 1),
                    )
                ps.append(p)
            return ps

        def emit_elu(psum, tag, out_dtype):
            """elu(x)+1 = exp(-relu(-x)) + relu(x); psum -> sbuf tile."""
            rn = apool.tile([P, EH], FP32, tag=f"rn{tag}")
            nc.scalar.activation(rn[:], psum[:], AF.Relu, scale=-1.0)
            ex = apool.tile([P, EH], FP32, tag=f"ex{tag}")
            nc.scalar.activation(ex[:], rn[:], AF.Exp, scale=-1.0)
            rp = apool.tile([P, EH], FP32, tag=f"rp{tag}")
            nc.scalar.activation(rp[:], psum[:], AF.Relu)
            o = apool.tile([P, EH], out_dtype, tag=f"in{tag}")
            nc.vector.tensor_add(o[:], ex[:], rp[:])
            return o

        def emit_stage1(xT, s0):
            """Everything up to (and incl.) k_in/kv_in/q_in for one block."""
            psq, psk, psv = emit_proj(xT, s0)
            q_in = emit_elu(psq, "q", FP32)
            k_in = emit_elu(psk, "k", FP16)
            kv_in = apool.tile([P, EH], FP16, tag="kv")
            nc.vector.tensor_mul(kv_in[:], k_in[:], psv[:])
            return q_in, k_in, kv_in

        prev_tail = [None, None]  # sbuf [P, EH] tiles holding rows 96:128

        def emit_stage2(state, s_global, n):
            """Cumsums + combine + store for block index n."""
            q_in, k_in, kv_in = state
            first = n == 0
            # block-local inclusive cumsum + carry broadcast (carry = row 127
            # of the previous block's inclusive cumsum, selected from its
            # copied tail rows by the K=32 selector matmul)
            ck = pc.tile([P, EH], FP32, tag="cum")
            nc.tensor.matmul(
                ck[:], lhsT=tri_t[:], rhs=k_in[:], start=True, stop=first
            )
            if not first:
                nc.tensor.matmul(
                    ck[:], lhsT=sel_t[96:P, :], rhs=prev_tail[0][96:P, :],
                    start=False, stop=True, tile_position=(96, 0),
                )
            ckv = pc.tile([P, EH], FP32, tag="cum")
            nc.tensor.matmul(
                ckv[:], lhsT=tri_t[:], rhs=kv_in[:], start=True, stop=first
            )
            if not first:
                nc.tensor.matmul(
                    ckv[:], lhsT=sel_t[96:P, :], rhs=prev_tail[1][96:P, :],
                    start=False, stop=True, tile_position=(96, 0),
                )
            # save this block's inclusive-cumsum tail rows for the next block
            tk = cpool.tile([P, EH], FP16, tag="ck")
            nc.scalar.copy(tk[96:P, :], ck[96:P, :])
            tkv = cpool.tile([P, EH], FP16, tag="ckv")
            nc.scalar.copy(tkv[96:P, :], ckv[96:P, :])
            prev_tail[0], prev_tail[1] = tk, tkv
            # out = q_in * (kv_in + ckv) / ck
            den = apool.tile([P, EH], FP32, tag="den")
            nc.vector.reciprocal(den[:], ck[:])
            num = apool.tile([P, EH], FP32, tag="num")
            nc.vector.tensor_add(num[:], ckv[:], kv_in[:])
            t1 = apool.tile([P, EH], FP32, tag="t1")
            nc.vector.tensor_mul(t1[:], q_in[:], num[:])
            ot = opool.tile([P, EH], FP32, tag="ot")
            nc.vector.tensor_mul(ot[:], t1[:], den[:])
            nc.sync.dma_start(out=out[s_global : s_global + P, :], in_=ot[:])

        # --- main loop: 1-block software-pipeline skew so the cumsum
        # matmuls of block i-1 fill PE behind the projections of block i ---
        def main_body():
            from collections import deque
            pending = deque()  # (state, s_global, n) awaiting stage2
            nblk = 0
            for c in range(n_chunks):
                xT = []  # [input] -> [P, n_d, CHUNK] fp16 (pre-transposed on host)
                for idx, xd in enumerate((xq, xk, xv)):
                    t = xpool.tile([P, n_d, CHUNK], FP16, tag=f"x{idx}")
                    nc.sync.dma_start(
                        out=t[:],
                        in_=xd[:, c * CHUNK : (c + 1) * CHUNK].rearrange(
                            "(j p) s -> p j s", p=P
                        ),
                    )
                    xT.append(t)
                for b in range(blocks_per_chunk):
                    s_global = c * CHUNK + b * P
                    state = emit_stage1(xT, b * P)
                    pending.append((state, s_global, nblk))
                    nblk += 1
                    if len(pending) > skew:
                        emit_stage2(*pending.popleft())
            while pending:
                emit_stage2(*pending.popleft())

        if repeat == 1:
            main_body()
        else:
            with tc.For_i(0, repeat, 1):
                main_body()

    nc.compile()
    return nc


def _host_prep(v, k, q, Wq, Wk, Wv):
    """Build the 8 per-core input maps (x^T computed once per batch)."""
    tri_np = np.triu(np.ones((P, P), dtype=np.float16))
    sel_np = np.zeros((P, P), dtype=np.float16)
    sel_np[P - 1, :] = 1.0
    xT = {
        b: {
            n: np.ascontiguousarray(x[b].T.astype(np.float16))
            for n, x in (("xq", q), ("xk", k), ("xv", v))
        }
        for b in range(B)
    }
    in_maps = []
    for c in range(NCORES):
        b, h = c // 2, c % 2
        e0 = h * EH
        in_maps.append(
            {
                "xq": xT[b]["xq"],
                "xk": xT[b]["xk"],
                "xv": xT[b]["xv"],
                "wqt": np.ascontiguousarray(Wq.T[:, e0 : e0 + EH].astype(np.float16)),
                "wkt": np.ascontiguousarray(Wk.T[:, e0 : e0 + EH].astype(np.float16)),
                "wvt": np.ascontiguousarray(Wv.T[:, e0 : e0 + EH].astype(np.float16)),
                "tri": tri_np,
                "sel": sel_np,
            }
        )
    return in_maps


_NC_CACHE = None


def _get_nc():
    global _NC_CACHE
    if _NC_CACHE is None:
        _NC_CACHE = build_nc()
    return _NC_CACHE


def run_spmd(v, k, q, Wq, Wk, Wv, **kwargs):
    """Run on 8 cores; returns (assembled output [B,S,E] fp32, raw results)."""
    nc = _get_nc()
    in_maps = _host_prep(v, k, q, Wq, Wk, Wv)
    res = run_bass_kernel_spmd(nc, in_maps, core_ids=list(range(NCORES)), **kwargs)
    full = np.empty((B, S, E), dtype=np.float32)
    for c in range(NCORES):
        b, h = c // 2, c % 2
        full[b, :, h * EH : (h + 1) * EH] = res.results[c]["out"]
    return full, res


def kernel(v, k, q, Wq, Wk, Wv):
    v, k, q, Wq, Wk, Wv = (
        np.asarray(a, dtype=np.float32) for a in (v, k, q, Wq, Wk, Wv)
    )
    full, _ = run_spmd(v, k, q, Wq, Wk, Wv)
    return full
